# revision 3
# baseline (speedup 1.0000x reference)
"""Trainium2 Bass kernel for bidirectional GRU (nn_Bidirectional).

Model: y = BN2(concat([GRU_f(BN1(x@w_in)), rev(GRU_b(rev(BN1(x@w_in))))]) @ w_out)
Shapes: x [64, 512, 128], H=512, O=8.

Sharding: 8 cores = 4 batch shards x 2 directions. Every core runs the SAME
SPMD program on its own inputs; the backward direction is realized by feeding
time-reversed x and un-reversing the partial output on the host. The final
projection is split per-direction (y = hf @ Wo[:H] + hb @ Wo[H:] + bias) and
summed on the host, with both BatchNorms folded into per-feature scale/bias
(device) and into w_out (host).

Device program (all tensors in transposed [feature, batch] layout):
  A: h_bn.T = Identity(w_in.T @ x.T, scale=s1, bias=b1)          (PE + ACT)
  B: xp.T   = wx.T @ h_bn.T + bias   -> DRAM scratch, bf16       (PE + ACT)
  C: 512 sequential GRU steps; weight-stationary matmuls (wh tiles bf16,
     FWL), xp added into PSUM via identity-matmul, gates on ACT/DVE,
     y-projection accumulated in PSUM per 8-step chunk.
"""

import sys
from contextlib import ExitStack

import numpy as np
import ml_dtypes

if "/opt/trn_rl_repo" not in sys.path:
    sys.path.insert(0, "/opt/trn_rl_repo")

B, T, F, H, O = 64, 512, 128, 512, 8
EPS = 1e-3
NCORES = 8
BC = B // 4          # batch per core = 16
KT = H // 128        # 4 H-strips
MT = 3 * H // 128    # 12 output strips (z0..3, r0..3, h0..3)
TOK = T * BC         # 8192 tokens per core, time-major: tok = t*BC + b
CHUNK = 512          # tokens per phase-A/B psum chunk
NCH = TOK // CHUNK   # 16
SCH = 8              # recurrence steps per chunk
NSCH = T // SCH      # 64 chunks
SB = SCH * BC        # 128 tokens per recurrence chunk
BF16 = ml_dtypes.bfloat16

_cache = {}


def _build(has_bh: bool):
    import concourse.bass as bass
    import concourse.bacc as bacc
    import concourse.tile as tile
    import concourse.mybir as mybir

    dt = mybir.dt
    f32 = dt.float32
    bf = dt.bfloat16
    AF = mybir.ActivationFunctionType
    OP = mybir.AluOpType
    ds = bass.ds

    nc = bacc.Bacc("TRN2", target_bir_lowering=False, debug=False,
                   num_devices=NCORES)

    xT = nc.dram_tensor("xT", [F, TOK], bf, kind="ExternalInput").ap()
    w_in = nc.dram_tensor("w_in", [F, H], bf, kind="ExternalInput").ap()
    bn1s = nc.dram_tensor("bn1s", [128, KT], f32, kind="ExternalInput").ap()
    bn1b = nc.dram_tensor("bn1b", [128, KT], f32, kind="ExternalInput").ap()
    wx = nc.dram_tensor("wx", [128, KT, MT, 128], bf, kind="ExternalInput").ap()
    wh = nc.dram_tensor("wh", [128, KT, MT, 128], bf, kind="ExternalInput").ap()
    bxp = nc.dram_tensor("bxp", [128, MT], f32, kind="ExternalInput").ap()
    bhr = nc.dram_tensor("bhr", [128, KT], f32, kind="ExternalInput").ap()
    ident = nc.dram_tensor("ident", [128, 128], bf, kind="ExternalInput").ap()
    wo = nc.dram_tensor("wo", [128, KT, O], bf, kind="ExternalInput").ap()
    bo = nc.dram_tensor("bo", [O, 1], f32, kind="ExternalInput").ap()
    yT = nc.dram_tensor("yT", [O, TOK], f32, kind="ExternalOutput").ap()

    with tile.TileContext(nc) as tc, ExitStack() as ctx:
        consts = ctx.enter_context(tc.tile_pool(name="consts", bufs=1))
        big = ctx.enter_context(tc.tile_pool(name="big", bufs=1))
        stg = ctx.enter_context(tc.tile_pool(name="stg", bufs=3))
        gates = ctx.enter_context(tc.tile_pool(name="gates", bufs=2))
        psAB = ctx.enter_context(tc.tile_pool(name="psAB", bufs=2, space="PSUM"))
        psR = ctx.enter_context(tc.tile_pool(name="psR", bufs=2, space="PSUM"))
        psY = ctx.enter_context(tc.tile_pool(name="psY", bufs=2, space="PSUM"))
        dram = ctx.enter_context(tc.tile_pool(name="dram", bufs=1, space="DRAM"))

        # ---------- constants ----------
        win_sb = consts.tile([128, H], bf)
        nc.sync.dma_start(out=win_sb, in_=w_in)
        bn1s_sb = consts.tile([128, KT], f32)
        nc.sync.dma_start(out=bn1s_sb, in_=bn1s)
        bn1b_sb = consts.tile([128, KT], f32)
        nc.sync.dma_start(out=bn1b_sb, in_=bn1b)
        wx_sb = consts.tile([128, KT, MT, 128], bf)
        nc.sync.dma_start(out=wx_sb, in_=wx)
        bxp_sb = consts.tile([128, MT], f32)
        nc.sync.dma_start(out=bxp_sb, in_=bxp)
        wh_sb = consts.tile([128, KT, MT, 128], bf)
        nc.sync.dma_start(out=wh_sb, in_=wh)
        id_sb = consts.tile([128, 128], bf)
        nc.sync.dma_start(out=id_sb, in_=ident)
        wo_sb = consts.tile([128, KT, O], bf)
        nc.sync.dma_start(out=wo_sb, in_=wo)
        bo_sb = consts.tile([O, 1], f32)
        nc.sync.dma_start(out=bo_sb, in_=bo)
        bhr_sb = consts.tile([128, KT], f32)
        nc.sync.dma_start(out=bhr_sb, in_=bhr)

        xt_sb = big.tile([128, TOK], bf, tag="xt")
        nc.sync.dma_start(out=xt_sb, in_=xT)

        # ---------- phase A: h_bn.T [128, KT, TOK] ----------
        hbn = big.tile([128, KT, TOK], bf, tag="hbn")
        for c in range(NCH):
            sl = slice(CHUNK * c, CHUNK * (c + 1))
            for s in range(KT):
                ps = psAB.tile([128, CHUNK], f32, tag="pab")
                nc.tensor.matmul(ps, win_sb[:, 128 * s:128 * (s + 1)],
                                 xt_sb[:, sl], start=True, stop=True)
                nc.scalar.activation(hbn[:, s, sl], ps, AF.Identity,
                                     bias=bn1b_sb[:, s:s + 1],
                                     scale=bn1s_sb[:, s:s + 1])

        # ---------- phase B: xp.T -> DRAM [128, MT, TOK+pad] bf16 ----------
        xp_dr = dram.tile([128, MT, TOK + 2 * SB], bf)
        for c in range(NCH):
            sl = slice(CHUNK * c, CHUNK * (c + 1))
            for m in range(MT):
                ps = psAB.tile([128, CHUNK], f32, tag="pab")
                for k in range(KT):
                    nc.tensor.matmul(ps, wx_sb[:, k, m, :], hbn[:, k, sl],
                                     start=(k == 0), stop=(k == KT - 1))
                st = stg.tile([128, CHUNK], bf, tag="st")
                nc.scalar.activation(st, ps, AF.Identity,
                                     bias=bxp_sb[:, m:m + 1], scale=1.0)
                nc.sync.dma_start(out=xp_dr[:, m, sl], in_=st)

        # ---------- phase C: recurrence ----------
        hA = big.tile([128, KT * BC], bf, tag="hA")
        hB = big.tile([128, KT * BC], bf, tag="hB")
        nc.vector.memset(hA, 0.0)
        xpA = big.tile([128, MT, SB], bf, tag="xpA")
        xpB = big.tile([128, MT, SB], bf, tag="xpB")
        nc.sync.dma_start(out=xpA, in_=xp_dr[:, :, 0:SB])

        def step(xp_c, j, h_in, h_out, psy):
            psZR = psR.tile([128, 2, KT, BC], f32, tag="zr")
            psH = psR.tile([128, KT, BC], f32, tag="h")
            xps = xp_c[:, :, BC * j:BC * (j + 1)]  # [128, MT, BC]
            # h-gate recurrent part (no xp)
            for mi in range(4):
                for k in range(KT):
                    nc.tensor.matmul(psH[:, mi], wh_sb[:, k, 8 + mi, :],
                                     h_in[:, BC * k:BC * (k + 1)],
                                     start=(k == 0), stop=(k == KT - 1))
            # r-gate
            for mi in range(4):
                for k in range(KT):
                    nc.tensor.matmul(psZR[:, 1, mi], wh_sb[:, k, 4 + mi, :],
                                     h_in[:, BC * k:BC * (k + 1)],
                                     start=(k == 0), stop=(k == KT - 1))
            # z-gate
            for mi in range(4):
                for k in range(KT):
                    nc.tensor.matmul(psZR[:, 0, mi], wh_sb[:, k, mi, :],
                                     h_in[:, BC * k:BC * (k + 1)],
                                     start=(k == 0), stop=(k == KT - 1))
            # gates
            preR = gates.tile([128, KT * BC], bf, tag="preR")
            nc.vector.tensor_add(preR, psZR[:, 1], xps[:, 4:8])
            r_sb = gates.tile([128, KT * BC], bf, tag="r")
            nc.scalar.activation(r_sb, preR, AF.Sigmoid)
            t1 = gates.tile([128, KT * BC], bf, tag="t1")
            if has_bh:
                for s in range(KT):
                    nc.vector.scalar_tensor_tensor(
                        t1[:, BC * s:BC * (s + 1)], psH[:, s],
                        bhr_sb[:, s:s + 1], r_sb[:, BC * s:BC * (s + 1)],
                        OP.add, OP.mult)
            else:
                nc.vector.tensor_mul(t1, psH, r_sb)
            t2 = gates.tile([128, KT * BC], bf, tag="t2")
            nc.vector.tensor_add(t2, t1, xps[:, 8:12])
            hh = gates.tile([128, KT * BC], bf, tag="hh")
            nc.scalar.activation(hh, t2, AF.Tanh)
            dd = gates.tile([128, KT * BC], bf, tag="dd")
            nc.vector.tensor_sub(dd, h_in, hh)
            preZ = gates.tile([128, KT * BC], bf, tag="preZ")
            nc.vector.tensor_add(preZ, psZR[:, 0], xps[:, 0:4])
            z_sb = gates.tile([128, KT * BC], bf, tag="z")
            nc.scalar.activation(z_sb, preZ, AF.Sigmoid)
            ee = gates.tile([128, KT * BC], bf, tag="ee")
            nc.vector.tensor_mul(ee, z_sb, dd)
            nc.vector.tensor_add(h_out, ee, hh)
            # y-projection for this step
            for k in range(KT):
                nc.tensor.matmul(psy[:, j], wo_sb[:, k, :],
                                 h_out[:, BC * k:BC * (k + 1)],
                                 start=(k == 0), stop=(k == KT - 1))

        def chunk_steps(xp_c, psy):
            hs = [hA, hB]
            for j in range(SCH):
                step(xp_c, j, hs[j % 2], hs[(j + 1) % 2], psy)

        with tc.For_i(0, NSCH, 2,
                      hint_engines=(mybir.EngineType.PE,)) as i:
            nc.sync.dma_start(out=xpB, in_=xp_dr[:, :, ds((i + 1) * SB, SB)])
            psy_a = psY.tile([O, SCH, BC], f32, tag="y")
            chunk_steps(xpA, psy_a)
            yst_a = stg.tile([O, SB], f32, tag="yst")
            nc.scalar.activation(yst_a, psy_a, AF.Identity, bias=bo_sb,
                                 scale=1.0)
            nc.sync.dma_start(out=yT[:, ds(i * SB, SB)], in_=yst_a)

            nc.sync.dma_start(out=xpA, in_=xp_dr[:, :, ds((i + 2) * SB, SB)])
            psy_b = psY.tile([O, SCH, BC], f32, tag="y")
            chunk_steps(xpB, psy_b)
            yst_b = stg.tile([O, SB], f32, tag="yst")
            nc.scalar.activation(yst_b, psy_b, AF.Identity, bias=bo_sb,
                                 scale=1.0)
            nc.sync.dma_start(out=yT[:, ds((i + 1) * SB, SB)], in_=yst_b)

    nc.compile()
    return nc


def _get_program(has_bh: bool):
    key = ("prog", has_bh)
    if key not in _cache:
        _cache[key] = _build(has_bh)
    return _cache[key]


def _prep_core(x_shard, rev, w_in, s1, b1, wx, wh, bb, wo_half, bias_out):
    """Build the per-core input map (numpy, device layouts/dtypes)."""
    xs = x_shard[:, ::-1] if rev else x_shard          # [BC, T, F]
    xTc = np.ascontiguousarray(xs.transpose(2, 1, 0)).reshape(F, TOK)
    bias_xp = np.concatenate([bb[0, :2 * H] + bb[1, :2 * H], bb[0, 2 * H:]])
    return {
        "xT": xTc.astype(BF16),
        "w_in": w_in.astype(BF16),
        "bn1s": np.ascontiguousarray(s1.reshape(KT, 128).T.astype(np.float32)),
        "bn1b": np.ascontiguousarray(b1.reshape(KT, 128).T.astype(np.float32)),
        "wx": np.ascontiguousarray(
            wx.reshape(KT, 128, MT, 128).transpose(1, 0, 2, 3)).astype(BF16),
        "wh": np.ascontiguousarray(
            wh.reshape(KT, 128, MT, 128).transpose(1, 0, 2, 3)).astype(BF16),
        "bxp": np.ascontiguousarray(
            bias_xp.reshape(MT, 128).T.astype(np.float32)),
        "bhr": np.ascontiguousarray(
            bb[1, 2 * H:].reshape(KT, 128).T.astype(np.float32)),
        "ident": np.eye(128).astype(BF16),
        "wo": np.ascontiguousarray(
            wo_half.reshape(KT, 128, O).transpose(1, 0, 2)).astype(BF16),
        "bo": bias_out.reshape(O, 1).astype(np.float32),
    }


def kernel(x, w_in, b_in, g1, be1, m1, v1, wxf, whf, bf, wxb, whb, bb,
           w_out, b_out, g2, be2, m2, v2):
    from concourse.bass_utils import run_bass_kernel_spmd

    args = locals()
    np_in = {k: np.asarray(args[k], np.float32) for k in (
        "x", "w_in", "b_in", "g1", "be1", "m1", "v1", "wxf", "whf", "bf",
        "wxb", "whb", "bb", "w_out", "b_out", "g2", "be2", "m2", "v2")}

    s1 = np_in["g1"] / np.sqrt(np_in["v1"] + EPS)
    b1 = (np_in["b_in"] - np_in["m1"]) * s1 + np_in["be1"]
    s2 = np_in["g2"] / np.sqrt(np_in["v2"] + EPS)
    b2 = (np_in["b_out"] - np_in["m2"]) * s2 + np_in["be2"]
    Ws = np_in["w_out"] * s2[None, :]

    has_bh = bool(np.any(np_in["bf"][1, 2 * H:]) or np.any(np_in["bb"][1, 2 * H:]))
    nc = _get_program(has_bh)

    in_maps = []
    for c in range(NCORES):
        d, s = c // 4, c % 4
        shard = np_in["x"][BC * s:BC * (s + 1)]
        if d == 0:
            m = _prep_core(shard, False, np_in["w_in"], s1, b1,
                           np_in["wxf"], np_in["whf"], np_in["bf"],
                           Ws[:H], b2)
        else:
            m = _prep_core(shard, True, np_in["w_in"], s1, b1,
                           np_in["wxb"], np_in["whb"], np_in["bb"],
                           Ws[H:], np.zeros(O, np.float32))
        in_maps.append(m)

    res = run_bass_kernel_spmd(nc, in_maps, core_ids=list(range(NCORES)))
    outs = res.results

    y = np.zeros((B, T, O), np.float32)
    for s in range(4):
        yf = outs[s]["yT"].reshape(O, T, BC)
        yb = outs[4 + s]["yT"].reshape(O, T, BC)[:, ::-1]
        y[BC * s:BC * (s + 1)] = (yf + yb).transpose(2, 1, 0)
    return y


# revision 6
# speedup vs baseline: 284.9066x; 284.9066x over previous
"""Trainium2 Bass kernel for bidirectional GRU (nn_Bidirectional).

Model: y = BN2(concat([GRU_f(BN1(x@w_in)), rev(GRU_b(rev(BN1(x@w_in))))]) @ w_out)
Shapes: x [64, 512, 128], H=512, O=8.

Sharding: 8 cores = 4 batch shards x 2 directions. Every core runs the SAME
SPMD program on its own inputs; the backward direction is realized by feeding
time-reversed x and un-reversing the partial output on the host. The final
projection is split per-direction (y = hf @ Wo[:H] + hb @ Wo[H:] + bias) and
summed on the host, with both BatchNorms folded into per-feature scale/bias
(device) and into w_out (host).

Device program (all tensors in transposed [feature, batch] layout):
  A: h_bn.T = Identity(w_in.T @ x.T, scale=s1, bias=b1)          (PE + ACT)
  B: xp.T   = wx.T @ h_bn.T + bias   -> DRAM scratch, bf16       (PE + ACT)
  C: 512 sequential GRU steps; weight-stationary matmuls (wh tiles bf16,
     FWL), xp added into PSUM via identity-matmul, gates on ACT/DVE,
     y-projection accumulated in PSUM per 8-step chunk.
"""

import sys
from contextlib import ExitStack

import numpy as np
import ml_dtypes

if "/opt/trn_rl_repo" not in sys.path:
    sys.path.insert(0, "/opt/trn_rl_repo")

B, T, F, H, O = 64, 512, 128, 512, 8
EPS = 1e-3
NCORES = 8
BC = B // 4          # batch per core = 16
KT = H // 128        # 4 H-strips
MT = 3 * H // 128    # 12 output strips (z0..3, r0..3, h0..3)
TOK = T * BC         # 8192 tokens per core, time-major: tok = t*BC + b
CHUNK = 512          # tokens per phase-A/B psum chunk
NCH = TOK // CHUNK   # 16
SCH = 8              # recurrence steps per chunk
NSCH = T // SCH      # 64 chunks
SB = SCH * BC        # 128 tokens per recurrence chunk
BF16 = ml_dtypes.bfloat16

_cache = {}


def _build(has_bh: bool, loop_reps: int = 1):
    import concourse.bass as bass
    import concourse.bacc as bacc
    import concourse.tile as tile
    import concourse.mybir as mybir

    dt = mybir.dt
    f32 = dt.float32
    bf = dt.bfloat16
    AF = mybir.ActivationFunctionType
    OP = mybir.AluOpType
    ds = bass.ds

    nc = bacc.Bacc("TRN2", target_bir_lowering=False, debug=False,
                   num_devices=NCORES)

    xT = nc.dram_tensor("xT", [F, TOK], bf, kind="ExternalInput").ap()
    w_in = nc.dram_tensor("w_in", [F, H], bf, kind="ExternalInput").ap()
    bn1s = nc.dram_tensor("bn1s", [128, KT], f32, kind="ExternalInput").ap()
    bn1b = nc.dram_tensor("bn1b", [128, KT], f32, kind="ExternalInput").ap()
    wx = nc.dram_tensor("wx", [128, KT, MT, 128], bf, kind="ExternalInput").ap()
    wh = nc.dram_tensor("wh", [128, KT, MT, 128], bf, kind="ExternalInput").ap()
    bxp = nc.dram_tensor("bxp", [128, MT], f32, kind="ExternalInput").ap()
    bhr = nc.dram_tensor("bhr", [128, KT], f32, kind="ExternalInput").ap()
    ident = nc.dram_tensor("ident", [128, 128], bf, kind="ExternalInput").ap()
    wo = nc.dram_tensor("wo", [128, KT, O], bf, kind="ExternalInput").ap()
    bo = nc.dram_tensor("bo", [O, 1], f32, kind="ExternalInput").ap()
    yT = nc.dram_tensor("yT", [O, TOK], f32, kind="ExternalOutput").ap()

    with tile.TileContext(nc) as tc, ExitStack() as ctx:
        consts = ctx.enter_context(tc.tile_pool(name="consts", bufs=1))
        big = ctx.enter_context(tc.tile_pool(name="big", bufs=1))
        stg = ctx.enter_context(tc.tile_pool(name="stg", bufs=3))
        gates = ctx.enter_context(tc.tile_pool(name="gates", bufs=2))
        psAB = ctx.enter_context(tc.tile_pool(name="psAB", bufs=2, space="PSUM"))
        psR = ctx.enter_context(tc.tile_pool(name="psR", bufs=2, space="PSUM"))
        psY = ctx.enter_context(tc.tile_pool(name="psY", bufs=2, space="PSUM"))
        dram = ctx.enter_context(tc.tile_pool(name="dram", bufs=1, space="DRAM"))

        # ---------- constants ----------
        win_sb = consts.tile([128, H], bf)
        nc.sync.dma_start(out=win_sb, in_=w_in)
        bn1s_sb = consts.tile([128, KT], f32)
        nc.sync.dma_start(out=bn1s_sb, in_=bn1s)
        bn1b_sb = consts.tile([128, KT], f32)
        nc.sync.dma_start(out=bn1b_sb, in_=bn1b)
        wx_sb = consts.tile([128, KT, MT, 128], bf)
        nc.sync.dma_start(out=wx_sb, in_=wx)
        bxp_sb = consts.tile([128, MT], f32)
        nc.sync.dma_start(out=bxp_sb, in_=bxp)
        wh_sb = consts.tile([128, KT, MT, 128], bf)
        nc.sync.dma_start(out=wh_sb, in_=wh)
        id_sb = consts.tile([128, 128], bf)
        nc.sync.dma_start(out=id_sb, in_=ident)
        wo_sb = consts.tile([128, KT, O], bf)
        nc.sync.dma_start(out=wo_sb, in_=wo)
        bo_sb = consts.tile([O, 1], f32)
        nc.sync.dma_start(out=bo_sb, in_=bo)
        bhr_sb = consts.tile([128, KT], f32)
        nc.sync.dma_start(out=bhr_sb, in_=bhr)

        xt_sb = big.tile([128, TOK], bf, tag="xt")
        nc.sync.dma_start(out=xt_sb, in_=xT)

        # ---------- phase A: h_bn.T [128, KT, TOK] ----------
        hbn = big.tile([128, KT, TOK], bf, tag="hbn")
        for c in range(NCH):
            sl = slice(CHUNK * c, CHUNK * (c + 1))
            for s in range(KT):
                ps = psAB.tile([128, CHUNK], f32, tag="pab")
                nc.tensor.matmul(ps, win_sb[:, 128 * s:128 * (s + 1)],
                                 xt_sb[:, sl], start=True, stop=True)
                nc.scalar.activation(hbn[:, s, sl], ps, AF.Identity,
                                     bias=bn1b_sb[:, s:s + 1],
                                     scale=bn1s_sb[:, s:s + 1])

        # ---------- phase B: xp.T -> DRAM [128, MT, TOK+pad] bf16 ----------
        xp_dr = dram.tile([128, MT, TOK + 2 * SB], bf)
        for c in range(NCH):
            sl = slice(CHUNK * c, CHUNK * (c + 1))
            for m in range(MT):
                ps = psAB.tile([128, CHUNK], f32, tag="pab")
                for k in range(KT):
                    nc.tensor.matmul(ps, wx_sb[:, k, m, :], hbn[:, k, sl],
                                     start=(k == 0), stop=(k == KT - 1))
                st = stg.tile([128, CHUNK], bf, tag="st")
                nc.scalar.activation(st, ps, AF.Identity,
                                     bias=bxp_sb[:, m:m + 1], scale=1.0)
                nc.sync.dma_start(out=xp_dr[:, m, sl], in_=st)

        # ---------- phase C: recurrence ----------
        hA = big.tile([128, KT * BC], bf, tag="hA")
        hB = big.tile([128, KT * BC], bf, tag="hB")
        nc.vector.memset(hA, 0.0)
        xpA = big.tile([128, MT, SB], bf, tag="xpA")
        xpB = big.tile([128, MT, SB], bf, tag="xpB")

        def step(xp_c, j, h_in, h_out, psy):
            psZR = psR.tile([128, 2, KT, BC], f32, tag="zr")
            psH = psR.tile([128, KT, BC], f32, tag="h")
            xps = xp_c[:, :, BC * j:BC * (j + 1)]  # [128, MT, BC]
            # h-gate recurrent part (no xp)
            for mi in range(4):
                for k in range(KT):
                    nc.tensor.matmul(psH[:, mi], wh_sb[:, k, 8 + mi, :],
                                     h_in[:, BC * k:BC * (k + 1)],
                                     start=(k == 0), stop=(k == KT - 1))
            # r-gate
            for mi in range(4):
                for k in range(KT):
                    nc.tensor.matmul(psZR[:, 1, mi], wh_sb[:, k, 4 + mi, :],
                                     h_in[:, BC * k:BC * (k + 1)],
                                     start=(k == 0), stop=(k == KT - 1))
            # z-gate
            for mi in range(4):
                for k in range(KT):
                    nc.tensor.matmul(psZR[:, 0, mi], wh_sb[:, k, mi, :],
                                     h_in[:, BC * k:BC * (k + 1)],
                                     start=(k == 0), stop=(k == KT - 1))
            # gates
            preR = gates.tile([128, KT * BC], bf, tag="preR")
            nc.vector.tensor_add(preR, psZR[:, 1], xps[:, 4:8])
            r_sb = gates.tile([128, KT * BC], bf, tag="r")
            nc.scalar.activation(r_sb, preR, AF.Sigmoid)
            t1 = gates.tile([128, KT * BC], bf, tag="t1")
            if has_bh:
                for s in range(KT):
                    nc.vector.scalar_tensor_tensor(
                        t1[:, BC * s:BC * (s + 1)], psH[:, s],
                        bhr_sb[:, s:s + 1], r_sb[:, BC * s:BC * (s + 1)],
                        OP.add, OP.mult)
            else:
                nc.vector.tensor_mul(t1, psH, r_sb)
            t2 = gates.tile([128, KT * BC], bf, tag="t2")
            nc.vector.tensor_add(t2, t1, xps[:, 8:12])
            hh = gates.tile([128, KT * BC], bf, tag="hh")
            nc.scalar.activation(hh, t2, AF.Tanh)
            dd = gates.tile([128, KT * BC], bf, tag="dd")
            nc.vector.tensor_sub(dd, h_in, hh)
            preZ = gates.tile([128, KT * BC], bf, tag="preZ")
            nc.vector.tensor_add(preZ, psZR[:, 0], xps[:, 0:4])
            z_sb = gates.tile([128, KT * BC], bf, tag="z")
            nc.scalar.activation(z_sb, preZ, AF.Sigmoid)
            ee = gates.tile([128, KT * BC], bf, tag="ee")
            nc.vector.tensor_mul(ee, z_sb, dd)
            nc.vector.tensor_add(h_out, ee, hh)
            # y-projection for this step
            for k in range(KT):
                nc.tensor.matmul(psy[:, j], wo_sb[:, k, :],
                                 h_out[:, BC * k:BC * (k + 1)],
                                 start=(k == 0), stop=(k == KT - 1))

        def chunk_steps(xp_c, psy):
            hs = [hA, hB]
            for j in range(SCH):
                step(xp_c, j, hs[j % 2], hs[(j + 1) % 2], psy)

        for _rep in range(loop_reps):
            nc.sync.dma_start(out=xpA, in_=xp_dr[:, :, 0:SB])
            with tc.For_i(0, NSCH, 2,
                          hint_engines=(mybir.EngineType.PE,)) as i:
                nc.sync.dma_start(out=xpB,
                                  in_=xp_dr[:, :, ds((i + 1) * SB, SB)])
                psy_a = psY.tile([O, SCH, BC], f32, tag="y")
                chunk_steps(xpA, psy_a)
                yst_a = stg.tile([O, SB], f32, tag="yst")
                nc.scalar.activation(yst_a, psy_a, AF.Identity, bias=bo_sb,
                                     scale=1.0)
                nc.sync.dma_start(out=yT[:, ds(i * SB, SB)], in_=yst_a)

                nc.sync.dma_start(out=xpA,
                                  in_=xp_dr[:, :, ds((i + 2) * SB, SB)])
                psy_b = psY.tile([O, SCH, BC], f32, tag="y")
                chunk_steps(xpB, psy_b)
                yst_b = stg.tile([O, SB], f32, tag="yst")
                nc.scalar.activation(yst_b, psy_b, AF.Identity, bias=bo_sb,
                                     scale=1.0)
                nc.sync.dma_start(out=yT[:, ds((i + 1) * SB, SB)], in_=yst_b)

    nc.compile()
    return nc


def _get_program(has_bh: bool):
    key = ("prog", has_bh)
    if key not in _cache:
        _cache[key] = _build(has_bh)
    return _cache[key]


def _prep_core(x_shard, rev, w_in, s1, b1, wx, wh, bb, wo_half, bias_out):
    """Build the per-core input map (numpy, device layouts/dtypes)."""
    xs = x_shard[:, ::-1] if rev else x_shard          # [BC, T, F]
    xTc = np.ascontiguousarray(xs.transpose(2, 1, 0)).reshape(F, TOK)
    bias_xp = np.concatenate([bb[0, :2 * H] + bb[1, :2 * H], bb[0, 2 * H:]])
    return {
        "xT": xTc.astype(BF16),
        "w_in": w_in.astype(BF16),
        "bn1s": np.ascontiguousarray(s1.reshape(KT, 128).T.astype(np.float32)),
        "bn1b": np.ascontiguousarray(b1.reshape(KT, 128).T.astype(np.float32)),
        "wx": np.ascontiguousarray(
            wx.reshape(KT, 128, MT, 128).transpose(1, 0, 2, 3)).astype(BF16),
        "wh": np.ascontiguousarray(
            wh.reshape(KT, 128, MT, 128).transpose(1, 0, 2, 3)).astype(BF16),
        "bxp": np.ascontiguousarray(
            bias_xp.reshape(MT, 128).T.astype(np.float32)),
        "bhr": np.ascontiguousarray(
            bb[1, 2 * H:].reshape(KT, 128).T.astype(np.float32)),
        "ident": np.eye(128).astype(BF16),
        "wo": np.ascontiguousarray(
            wo_half.reshape(KT, 128, O).transpose(1, 0, 2)).astype(BF16),
        "bo": bias_out.reshape(O, 1).astype(np.float32),
    }


def kernel(x, w_in, b_in, g1, be1, m1, v1, wxf, whf, bf, wxb, whb, bb,
           w_out, b_out, g2, be2, m2, v2):
    from concourse.bass_utils import run_bass_kernel_spmd

    args = locals()
    np_in = {k: np.asarray(args[k], np.float32) for k in (
        "x", "w_in", "b_in", "g1", "be1", "m1", "v1", "wxf", "whf", "bf",
        "wxb", "whb", "bb", "w_out", "b_out", "g2", "be2", "m2", "v2")}

    s1 = np_in["g1"] / np.sqrt(np_in["v1"] + EPS)
    b1 = (np_in["b_in"] - np_in["m1"]) * s1 + np_in["be1"]
    s2 = np_in["g2"] / np.sqrt(np_in["v2"] + EPS)
    b2 = (np_in["b_out"] - np_in["m2"]) * s2 + np_in["be2"]
    Ws = np_in["w_out"] * s2[None, :]

    has_bh = bool(np.any(np_in["bf"][1, 2 * H:]) or np.any(np_in["bb"][1, 2 * H:]))
    nc = _get_program(has_bh)

    in_maps = []
    for c in range(NCORES):
        d, s = c // 4, c % 4
        shard = np_in["x"][BC * s:BC * (s + 1)]
        if d == 0:
            m = _prep_core(shard, False, np_in["w_in"], s1, b1,
                           np_in["wxf"], np_in["whf"], np_in["bf"],
                           Ws[:H], b2)
        else:
            m = _prep_core(shard, True, np_in["w_in"], s1, b1,
                           np_in["wxb"], np_in["whb"], np_in["bb"],
                           Ws[H:], np.zeros(O, np.float32))
        in_maps.append(m)

    res = run_bass_kernel_spmd(nc, in_maps, core_ids=list(range(NCORES)))
    outs = res.results

    y = np.zeros((B, T, O), np.float32)
    for s in range(4):
        yf = outs[s]["yT"].reshape(O, T, BC)
        yb = outs[4 + s]["yT"].reshape(O, T, BC)[:, ::-1]
        y[BC * s:BC * (s + 1)] = (yf + yb).transpose(2, 1, 0)
    return y
